# revision 73
# baseline (speedup 1.0000x reference)
"""BoundaryLoss kernel for Trainium2 (8 NeuronCores, data-parallel over batch).

Problem: for each (batch, waypoint), find the nearest boundary point (argmin
over N=4096 of euclidean distance), take dot(waypoint - closest_pt,
closest_normal), apply exp_relu, and mean over everything.

Key structure (per core: 4 batches; per batch 2 chunks of 128 waypoints):
  - Scores s[w, n] = w.b_n - 0.5||b_n||^2 (argmax_n s == argmin_n dist) are
    computed with float32r matmuls (1 PE cycle/row vs 4 for fp32). f32r
    rounds inputs to 12 mantissa bits, so every operand is Dekker-split into
    hi+lo halves host-side; the K dim grows 6->15 rows and the products
    reconstruct full fp32 scores exactly (PE cost depends only on moving
    columns, so this is free).
  - Level-1 fold (4096->2048) runs on PE+ACT instead of DVE/Pool:
    max(s0,s1) = s0 + relu(s1-s0). One matmul computes diff = s0-s1 from
    host-precomputed difference columns, ACT applies relu(-diff), and an
    identity matmul accumulates the relu back into s0's PSUM banks. The
    identity matmul re-rounds the relu values to 12 bits, so boundary points
    are Morton-ordered host-side and paired (q, q+2048) with near neighbors,
    keeping |s1-s0| small and the rounding harmless (~6e-3 rel on the loss).
  - Vector engines can read only one PSUM operand per op and gpsimd cannot
    touch PSUM at all, so ACT copies the right half of m1 to SBUF; DVE folds
    2048->512, then max8 + max_index give the fold position p. The 8 aliases
    {p+512k} are fetched with one indirect DMA per tile from a per-batch
    DRAM table and re-scored exactly with vector ops (is_ge tree prefers
    the lowest alias on ties).
  - exp_relu + row-sum on device; host sums the 8 cores' [128] partials.
"""

import numpy as np

import concourse.bass as bass
import concourse.bacc as bacc
import concourse.bass_utils as bass_utils
import concourse.mybir as mybir
from concourse.tile import TileContext

B, W, N, D = 32, 256, 4096, 3
N_CORES = 8
BPC = B // N_CORES          # batches per core = 4
WCHUNKS = W // 128          # waypoint chunks of 128 per batch
TILES = BPC * WCHUNKS       # 8 (batch, wchunk) tiles per core
HALF = N // 2               # 2048: level-1 fold output length
QUART = N // 4              # 1024: level-2 output
L = N // 8                  # 512: folded length fed to max8
NAL = 8                     # aliases per fold position
ROWF = 8 * NAL              # floats per gather-table row (8 per alias)

# big16 column layout: per batch b: rb at [b*N, b*N+N); rbd after all rb;
# wa after that.
RB0 = 0
RBD0 = BPC * N
WA0 = RBD0 + BPC * HALF
BIG16 = WA0 + BPC * W

F32 = mybir.dt.float32
F32R = mybir.dt.float32r
I32 = mybir.dt.int32
U32 = mybir.dt.uint32
ALU = mybir.AluOpType
ACTF = mybir.ActivationFunctionType
AX = mybir.AxisListType


def build_bass():
    nc = bacc.Bacc()

    # ---- DRAM I/O (host-packed; see make_in_maps) ----
    # big16 part 1: batch-0 slice [16, N+HALF+...]; we split the load into
    # "what tile 0 needs" and "the rest" to shorten the critical lead-in.
    big16 = nc.dram_tensor("big16", [16, BIG16], F32R, kind="ExternalInput")
    big128 = nc.dram_tensor("big128", [128, 128 + TILES * D], F32R,
                            kind="ExternalInput")
    gsrcs = [nc.dram_tensor(f"gsrc{b}", [L, ROWF], F32, kind="ExternalInput")
             for b in range(BPC)]
    res = nc.dram_tensor("res", [128, 1], F32, kind="ExternalOutput")

    with TileContext(nc) as tc:
        with (
            tc.tile_pool(name="const", bufs=1) as cpool,
            tc.tile_pool(name="work", bufs=4) as wpool,
            tc.tile_pool(name="small", bufs=4) as spool,
            tc.tile_pool(name="psum", bufs=1, space="PSUM") as psumpool,
        ):
            # ---- p-state ramp: keep PE busy on a zeroed dummy tile while
            # the input DMAs land, so real matmuls start at full clock ----
            z = cpool.tile([4, 512], mybir.dt.bfloat16)
            nc.vector.memset(z[:], 0.0)

            # ---- input loads: first what tile 0 needs, then the rest ----
            sb16 = cpool.tile([16, BIG16], F32R)
            sb128 = cpool.tile([128, 128 + TILES * D], F32R)
            nc.sync.dma_start(out=sb16[:, RBD0:RBD0 + HALF],
                              in_=big16[:, RBD0:RBD0 + HALF])
            nc.sync.dma_start(out=sb16[:, WA0:BIG16],
                              in_=big16[:, WA0:BIG16])
            nc.sync.dma_start(out=sb128[:], in_=big128[:])
            nc.sync.dma_start(out=sb16[:, 0:N], in_=big16[:, 0:N])
            nc.sync.dma_start(out=sb16[:, N:RBD0], in_=big16[:, N:RBD0])
            nc.sync.dma_start(out=sb16[:, RBD0 + HALF:WA0],
                              in_=big16[:, RBD0 + HALF:WA0])

            wat = sb16[:, WA0:WA0 + BPC * W]
            idt = sb128[:, 0:128]
            wp_all = sb128[:, 128:].bitcast(F32).rearrange(
                "p (t d) -> p t d", d=D)

            def rbv(b):
                return sb16[:, RB0 + b * N:RB0 + (b + 1) * N]

            def rbdv(b):
                return sb16[:, RBD0 + b * HALF:RBD0 + (b + 1) * HALF]

            gall = cpool.tile([128, TILES, ROWF], F32)
            dots = cpool.tile([128, TILES], F32)

            # PSUM (8 banks): Ya/Yb double-buffered across tiles. Each tile
            # first writes diff = s0-s1 into its Ya'/Yb' banks, ACT reads
            # them into relu(-diff), then s0 overwrites the same banks and
            # identity matmuls accumulate the relu on top.
            Ya = [psumpool.tile([128, QUART], F32, tag=f"Ya{h}",
                                name=f"Ya{h}") for h in range(2)]
            Yb = [psumpool.tile([128, QUART], F32, tag=f"Yb{h}",
                                name=f"Yb{h}") for h in range(2)]
            X = Ya[0]

            # ---- PE warm-up: dummy chain on the zeroed tile burns through
            # the p-state ramp while input DMAs land; then touch each input
            # tile to pre-observe its DMA semaphore ----
            for k in range(4):
                nc.tensor.matmul(out=X[0:1, 0:512], lhsT=z[:, 0:1],
                                 rhs=z[:], start=True, stop=True)
            nc.tensor.matmul(out=X[0:1, 0:2], lhsT=wat[:, 0:1],
                             rhs=wat[:, 0:2], start=True, stop=True)
            nc.tensor.matmul(out=X[0:1, 2:4], lhsT=idt[:, 0:1],
                             rhs=idt[:, 0:2], start=True, stop=True)
            nc.tensor.matmul(out=X[0:1, 4:6], lhsT=wat[:, 0:1],
                             rhs=rbdv(0)[:, 0:2], start=True, stop=True)
            nc.tensor.matmul(out=X[0:1, 6:8], lhsT=wat[:, 0:1],
                             rhs=rbv(0)[:, 0:2], start=True, stop=True)

            # ---- main loop, software-pipelined: iteration t issues the
            # matmul/relu front-end for tile t and the fold/select back-end
            # for tile t-1, so no engine queue blocks on a long dependency.
            avs = [None] * TILES

            def lhsT_of(t):
                b, wc = divmod(t, WCHUNKS)
                return wat[:, b * W + 128 * wc:b * W + 128 * (wc + 1)]

            def fr_diff(t):
                # diff = s0 - s1 into the tile's own banks, then
                # a = relu(-diff) on ACT. Issued one tile ahead so the
                # s/ident matmuls of the previous tile never wait on it.
                b, _ = divmod(t, WCHUNKS)
                lhsT = lhsT_of(t)
                ya, yb = Ya[t % 2], Yb[t % 2]
                a = wpool.tile([128, HALF], F32R, tag="a", name=f"a{t}")
                for k in range(2):
                    sl = slice(512 * k, 512 * (k + 1))
                    nc.tensor.matmul(out=ya[:, sl], lhsT=lhsT,
                                     rhs=rbdv(b)[:, 512 * k:512 * (k + 1)],
                                     start=True, stop=True)
                for k in range(2):
                    sl = slice(512 * k, 512 * (k + 1))
                    nc.tensor.matmul(out=yb[:, sl], lhsT=lhsT,
                                     rhs=rbdv(b)[:, 1024 + 512 * k:
                                                 1024 + 512 * (k + 1)],
                                     start=True, stop=True)
                nc.scalar.activation(out=a[:, 0:1024], in_=ya[:],
                                     func=ACTF.Relu, scale=-1.0)
                nc.scalar.activation(out=a[:, 1024:2048], in_=yb[:],
                                     func=ACTF.Relu, scale=-1.0)
                avs[t] = a

            def fr_main(t):
                # s0 overwrites the banks; identity matmuls add the relu.
                b, _ = divmod(t, WCHUNKS)
                lhsT = lhsT_of(t)
                ya, yb = Ya[t % 2], Yb[t % 2]
                a = avs[t]
                for k in range(2):
                    sl = slice(512 * k, 512 * (k + 1))
                    nc.tensor.matmul(out=ya[:, sl], lhsT=lhsT,
                                     rhs=rbv(b)[:, sl],
                                     start=True, stop=False)
                for k in range(2):
                    sl = slice(512 * k, 512 * (k + 1))
                    nc.tensor.matmul(out=ya[:, sl], lhsT=idt[:],
                                     rhs=a[:, sl], start=False, stop=True)
                for k in range(2):
                    sl = slice(512 * k, 512 * (k + 1))
                    nc.tensor.matmul(out=yb[:, sl], lhsT=lhsT,
                                     rhs=rbv(b)[:, 1024 + 512 * k:
                                                 1024 + 512 * (k + 1)],
                                     start=True, stop=False)
                for k in range(2):
                    sl = slice(512 * k, 512 * (k + 1))
                    nc.tensor.matmul(out=yb[:, sl], lhsT=idt[:],
                                     rhs=a[:, 1024 + 512 * k:
                                            1024 + 512 * (k + 1)],
                                     start=False, stop=True)
            cs = [None] * TILES

            def copy_out(t):
                # m1-right to SBUF for the fold (one-PSUM-operand rule)
                yb = Yb[t % 2]
                c = wpool.tile([128, QUART], F32, tag="c", name=f"c{t}")
                nc.scalar.copy(out=c[:], in_=yb[:])
                cs[t] = c

            def back(t):
                b, _ = divmod(t, WCHUNKS)
                ya = Ya[t % 2]
                c = cs[t]
                # fold position p aliases: m1[p], m1[p+512] (Ya),
                # m1[p+1024], m1[p+1536] (c). Ya folds via a single-input
                # strided reduce that depends only on its own idents — not
                # on c — so Ya frees early for the next same-parity tile.
                t2 = wpool.tile([128, L], F32, tag="t2", name=f"t2_{t}")
                nc.vector.tensor_reduce(
                    out=t2[:], in_=ya[:].rearrange("p (a q) -> p q a", a=2),
                    axis=AX.X, op=ALU.max)
                t1 = wpool.tile([128, L], F32, tag="t1", name=f"t1_{t}")
                nc.vector.tensor_tensor(out=t1[:], in0=c[:, 0:512],
                                        in1=c[:, 512:1024], op=ALU.max)
                f3 = wpool.tile([128, L], F32, tag="f3", name=f"f3_{t}")
                nc.vector.tensor_tensor(out=f3[:], in0=t1[:], in1=t2[:],
                                        op=ALU.max)
                v8 = spool.tile([128, 8], F32, tag="v8", bufs=6)
                nc.vector.max(out=v8[:], in_=f3[:])
                i8 = spool.tile([128, 8], U32, tag="i8", bufs=6,
                                name=f"i8_{t}")
                nc.vector.max_index(out=i8[:], in_max=v8[:], in_values=f3[:])
                nc.gpsimd.indirect_dma_start(
                    out=gall[:, t, :], out_offset=None, in_=gsrcs[b][:],
                    in_offset=bass.IndirectOffsetOnAxis(
                        ap=i8[:, 0:1].bitcast(I32), axis=0))

            # ---- verify: among the 8 aliases pick the true nearest and
            # emit dot(w - b, n); gall row per alias: [bx by bz hb nx ny nz c]
            def verify(t0, t1, dve_only=False):
                n = t1 - t0
                eng = nc.vector if dve_only else nc.gpsimd
                g = gall[:, t0:t1, :].rearrange("p t (a f) -> p t a f", f=8)
                wpv = wp_all[:, t0:t1, :].unsqueeze(2).broadcast_to(
                    [128, n, NAL, D])
                pr = cpool.tile([128, n, NAL, D], F32, tag=f"pr{t0}",
                                name=f"pr{t0}")
                eng.tensor_tensor(out=pr[:], in0=wpv,
                                  in1=g[:, :, :, 0:3], op=ALU.mult)
                wb = cpool.tile([128, n, NAL], F32, tag=f"wb{t0}",
                                name=f"wb{t0}")
                nc.vector.tensor_reduce(out=wb[:], in_=pr[:], axis=AX.X,
                                        op=ALU.add)
                pr2 = cpool.tile([128, n, NAL, D], F32, tag=f"pr2{t0}",
                                 name=f"pr2{t0}")
                eng.tensor_tensor(out=pr2[:], in0=wpv,
                                  in1=g[:, :, :, 4:7], op=ALU.mult)
                wn = cpool.tile([128, n, NAL], F32, tag=f"wn{t0}",
                                name=f"wn{t0}")
                nc.vector.tensor_reduce(out=wn[:], in_=pr2[:], axis=AX.X,
                                        op=ALU.add)
                sc = cpool.tile([128, n, NAL], F32, tag=f"sc{t0}",
                                name=f"sc{t0}")
                eng.tensor_tensor(out=sc[:], in0=wb[:],
                                  in1=g[:, :, :, 3], op=ALU.subtract)
                dt = cpool.tile([128, n, NAL], F32, tag=f"dt{t0}",
                                name=f"dt{t0}")
                eng.tensor_tensor(out=dt[:], in0=wn[:],
                                  in1=g[:, :, :, 7], op=ALU.subtract)
                # pairwise tournament, lower alias wins ties
                scur, dcur = sc, dt
                wdt = NAL
                while wdt > 1:
                    wdt //= 2
                    ev = (slice(None), slice(None), slice(0, 2 * wdt, 2))
                    od = (slice(None), slice(None), slice(1, 2 * wdt, 2))
                    m = cpool.tile([128, n, wdt], U32, tag=f"m{t0}_{wdt}",
                                   name=f"m{t0}_{wdt}")
                    nc.vector.tensor_tensor(out=m[:], in0=scur[ev],
                                            in1=scur[od], op=ALU.is_ge)
                    s2 = cpool.tile([128, n, wdt], F32, tag=f"s{t0}_{wdt}",
                                    name=f"s{t0}_{wdt}")
                    nc.vector.tensor_tensor(out=s2[:], in0=scur[ev],
                                            in1=scur[od], op=ALU.max)
                    d2 = cpool.tile([128, n, wdt], F32, tag=f"d{t0}_{wdt}",
                                    name=f"d{t0}_{wdt}")
                    nc.vector.tensor_copy(d2[:], dcur[od])
                    nc.vector.copy_predicated(d2[:], m[:], dcur[ev])
                    scur, dcur = s2, d2
                nc.vector.tensor_copy(dots[:, t0:t1], dcur[:, :, 0])

            # main loop with verifies interleaved so they overlap the
            # folds in each engine's queue instead of piling up at the end
            fr_diff(0)
            for t in range(TILES):
                fr_main(t)
                copy_out(t)
                if t > 0:
                    back(t - 1)
                if t + 1 < TILES:
                    fr_diff(t + 1)
            def fast_verify(t):
                # single-tile verify via max + mask (short serial chain):
                # dot = sum_k dot_k * [score_k >= max score] (exact fp32
                # ties between distinct candidates are measure-zero)
                g = gall[:, t, :].rearrange("p (a f) -> p a f", f=8)
                wpv = wp_all[:, t, :].unsqueeze(1).broadcast_to(
                    [128, NAL, D])
                pr = cpool.tile([128, NAL, D], F32, tag=f"fpr{t}",
                                name=f"fpr{t}")
                nc.vector.tensor_tensor(out=pr[:], in0=wpv,
                                        in1=g[:, :, 0:3], op=ALU.mult)
                wb = cpool.tile([128, NAL], F32, tag=f"fwb{t}",
                                name=f"fwb{t}")
                nc.vector.tensor_reduce(out=wb[:], in_=pr[:], axis=AX.X,
                                        op=ALU.add)
                pr2 = cpool.tile([128, NAL, D], F32, tag=f"fpr2{t}",
                                 name=f"fpr2{t}")
                nc.vector.tensor_tensor(out=pr2[:], in0=wpv,
                                        in1=g[:, :, 4:7], op=ALU.mult)
                wn = cpool.tile([128, NAL], F32, tag=f"fwn{t}",
                                name=f"fwn{t}")
                nc.vector.tensor_reduce(out=wn[:], in_=pr2[:], axis=AX.X,
                                        op=ALU.add)
                sc = cpool.tile([128, NAL], F32, tag=f"fsc{t}",
                                name=f"fsc{t}")
                nc.vector.tensor_tensor(out=sc[:], in0=wb[:],
                                        in1=g[:, :, 3], op=ALU.subtract)
                dt = cpool.tile([128, NAL], F32, tag=f"fdt{t}",
                                name=f"fdt{t}")
                nc.vector.tensor_tensor(out=dt[:], in0=wn[:],
                                        in1=g[:, :, 7], op=ALU.subtract)
                mx = cpool.tile([128, 1], F32, tag=f"fmx{t}",
                                name=f"fmx{t}")
                nc.vector.tensor_reduce(out=mx[:], in_=sc[:], axis=AX.X,
                                        op=ALU.max)
                msk = cpool.tile([128, NAL], F32, tag=f"fmk{t}",
                                name=f"fmk{t}")
                nc.vector.tensor_scalar(out=msk[:], in0=sc[:],
                                        scalar1=mx[:, 0:1], scalar2=None,
                                        op0=ALU.is_ge)
                sel = cpool.tile([128, NAL], F32, tag=f"fsl{t}",
                                 name=f"fsl{t}")
                nc.vector.tensor_tensor(out=sel[:], in0=msk[:], in1=dt[:],
                                        op=ALU.mult)
                nc.vector.tensor_reduce(out=dots[:, t:t + 1], in_=sel[:],
                                        axis=AX.X, op=ALU.add)

            def exp_tail(t0, t1):
                # exp_relu(dots[t0:t1]) -> em1[t0:t1]
                nc.scalar.activation(out=e[:, t0:t1], in_=dots[:, t0:t1],
                                     func=ACTF.Exp, scale=0.5)
                nc.vector.tensor_scalar(out=em1[:, t0:t1], in0=e[:, t0:t1],
                                        scalar1=-1.0, scalar2=None,
                                        op0=ALU.add)
                nc.vector.tensor_scalar(out=gmask[:, t0:t1],
                                        in0=dots[:, t0:t1], scalar1=0.0,
                                        scalar2=None, op0=ALU.is_gt)
                nc.vector.copy_predicated(em1[:, t0:t1], gmask[:, t0:t1],
                                          dots[:, t0:t1])

            e = cpool.tile([128, TILES], F32)
            em1 = cpool.tile([128, TILES], F32)
            gmask = cpool.tile([128, TILES], U32)

            back(TILES - 1)
            verify(0, 4)
            verify(4, 7)
            exp_tail(0, 7)
            fast_verify(TILES - 1)
            exp_tail(7, 8)
            sums = cpool.tile([128, 1], F32)
            nc.vector.reduce_sum(out=sums[:], in_=em1[:], axis=AX.X)
            nc.sync.dma_start(out=res[:], in_=sums[:])

    nc.finalize()
    return nc


_NC_CACHE = None


def _get_nc():
    global _NC_CACHE
    if _NC_CACHE is None:
        _NC_CACHE = build_bass()
    return _NC_CACHE


def _split12(x):
    """Split fp32 array into hi (top 12 mantissa bits, f32r-exact) + lo."""
    x = np.asarray(x, dtype=np.float32)
    c = np.float32((1 << 12) + 1)
    t = (c * x).astype(np.float32)
    hi = (t - (t - x).astype(np.float32)).astype(np.float32)
    lo = (x - hi).astype(np.float32)
    return hi, lo


def _morton_perm(bp):
    """Order boundary points so consecutive points are spatial neighbors,
    then lay pairs out as (q, q+HALF)."""
    lo = bp.min(0)
    span = bp.max(0) - lo + 1e-9
    q = np.floor((bp - lo) / span * 31.999).astype(np.int64)
    code = np.zeros(bp.shape[0], dtype=np.int64)
    for i in range(5):
        for d in range(3):
            code |= ((q[:, d] >> i) & 1) << (3 * i + d)
    order = np.argsort(code, kind="stable")
    perm = np.empty(bp.shape[0], dtype=np.int64)
    perm[:HALF] = order[0::2]
    perm[HALF:] = order[1::2]
    return perm


def _pack16(bT, sq):
    """[16, cols] f32 block: rows bh, bl, bh, sh, sl, 0 from [3, cols]."""
    bh, bl = _split12(bT)
    sh, sl = _split12(sq)
    out = np.zeros((16, bT.shape[1]), dtype=np.float32)
    out[0:3] = bh
    out[3:6] = bl
    out[6:9] = bh
    out[9:12] = sh
    out[12:15] = sl
    return out


def make_in_maps(waypoints, boundarypoints, boundarynormals):
    waypoints = np.ascontiguousarray(waypoints, dtype=np.float32)
    boundarypoints = np.ascontiguousarray(boundarypoints, dtype=np.float32)
    boundarynormals = np.ascontiguousarray(boundarynormals, dtype=np.float32)
    in_maps = []
    for c in range(N_CORES):
        sl = slice(c * BPC, (c + 1) * BPC)
        wp_c = waypoints[sl]                      # [4, 256, 3]
        bp_c = boundarypoints[sl]                 # [4, 4096, 3]
        nrm_c = boundarynormals[sl]               # [4, 4096, 3]

        big16 = np.zeros((16, BIG16), dtype=np.float32)
        # lhsT rows: wh, wh, wl, -0.5 x6, 0
        wT = wp_c.transpose(0, 2, 1).reshape(BPC, D, W)
        for b in range(BPC):
            wh, wl = _split12(wT[b])
            blk = slice(WA0 + b * W, WA0 + (b + 1) * W)
            big16[0:3, blk] = wh
            big16[3:6, blk] = wh
            big16[6:9, blk] = wl
            big16[9:15, blk] = -0.5

        gsrcs = {}
        for b in range(BPC):
            perm = _morton_perm(bp_c[b])
            bpP = bp_c[b][perm]                   # [N, 3] permuted
            nrP = nrm_c[b][perm]
            sqP = (bpP * bpP).astype(np.float32)
            big16[:, RB0 + b * N:RB0 + (b + 1) * N] = _pack16(
                bpP.T.astype(np.float32), sqP.T.astype(np.float32))
            db = (bpP[:HALF] - bpP[HALF:]).astype(np.float32)
            ds = (sqP[:HALF] - sqP[HALF:]).astype(np.float32)
            big16[:, RBD0 + b * HALF:RBD0 + (b + 1) * HALF] = _pack16(
                db.T.astype(np.float32), ds.T.astype(np.float32))
            # gather table: row p, alias k at perm position p + L*k:
            # [bx by bz hb nx ny nz c]
            g = np.empty((L, ROWF), dtype=np.float32)
            for k in range(NAL):
                seg = slice(L * k, L * (k + 1))
                bseg = bpP[seg]
                nseg = nrP[seg]
                g[:, 8 * k:8 * k + 3] = bseg
                g[:, 8 * k + 3] = 0.5 * (bseg * bseg).sum(1, dtype=np.float32)
                g[:, 8 * k + 4:8 * k + 7] = nseg
                g[:, 8 * k + 7] = (bseg * nseg).sum(1, dtype=np.float32)
            gsrcs[f"gsrc{b}"] = np.ascontiguousarray(g)

        big128 = np.zeros((128, 128 + TILES * D), dtype=np.float32)
        big128[:, 0:128] = np.eye(128, dtype=np.float32)
        for t in range(TILES):
            b, wc = divmod(t, WCHUNKS)
            big128[:, 128 + t * D:128 + (t + 1) * D] = \
                wp_c[b, 128 * wc:128 * (wc + 1), :]

        in_maps.append({
            "big16": np.ascontiguousarray(big16),
            "big128": np.ascontiguousarray(big128),
            **gsrcs,
        })
    return in_maps


def run_on_device(waypoints, boundarypoints, boundarynormals, trace=False):
    nc = _get_nc()
    in_maps = make_in_maps(waypoints, boundarypoints, boundarynormals)
    out = bass_utils.run_bass_kernel_spmd(
        nc, in_maps, core_ids=list(range(N_CORES)), trace=trace)
    total = np.float64(0.0)
    for r in out.results:
        total += np.sum(r["res"], dtype=np.float64)
    value = np.float32(total / (B * W))
    return value, out


def kernel(waypoints, boundarypoints, boundarynormals):
    value, _ = run_on_device(waypoints, boundarypoints, boundarynormals)
    return np.asarray(value, dtype=np.float32)
